# revision 37
# baseline (speedup 1.0000x reference)
"""Trainium2 Bass kernel for nn_LHFA_76278619177511.

Fused transposed-attention block (LHFA):
  q = dwconv3(conv1x1(x, Wq), Wq_dw)   (same for k from y, v from z)
  attn = softmax(l2norm(q) @ l2norm(k)^T * temp)   per-head [32,32]
  out = Wproj @ (attn @ v) + Wfus @ [x;y;z]

Strategy (per core, pure data-parallel over batch B=8 on 8 cores):
  - fp8e4m3 DoubleRow matmuls (0.5 PE cycles per output row, two K-tiles
    per matmul) for the three 3x3+1x1 folded convs, the per-head Gram
    matrices, and the attention-output projection. The fusion path
    (Wfus @ [x;y;z]) stays bf16 -- it dominates the output magnitude, so
    it carries the accuracy; the fp8 attention path's error is diluted.
  - Pad tiles use 129-wide rows (left pad only; the next row's left-pad
    zero doubles as this row's right pad), so the walrus-legal stationary
    slot stride of 128 steps (+1 row, -1 col). With a single A|G tile per
    input (G = A shifted +1 row +1 col), all 9 depthwise taps fit in 3
    DoubleRow quads per conv; unused slots get zero weights.
  - attn @ v is folded into the projection: Wfused = Wproj*blockdiag(attn)
    is built on-chip with 2 tiny bf16 matmuls after softmax, cast to fp8,
    and pass 2 runs out = Wfused^T*v (1 DoubleRow matmul per output half)
    + bf16 fusion matmuls.
  - Scaling: conv weights x2048 (their entries are products of two 0.02-
    scale gaussians), Wproj x2048; output descaled by 2^-22 at the final
    PSUM->SBUF copy. q/k scale cancels in the L2 normalization.
"""

import numpy as np
import ml_dtypes

import bass_rust
import concourse.bass as bass
import concourse.mybir as mybir
from concourse import tile as tile_mod
from concourse.tile import TileContext
from concourse.vector_clock import ScopedClock
from concourse.bass_utils import run_bass_kernel_spmd

BF16 = mybir.dt.bfloat16
FP8 = mybir.dt.float8e4
F32 = mybir.dt.float32
NP8 = ml_dtypes.float8_e4m3

C = 64          # input channels
DIM = 256       # q/k/v channels
HEADS = 8
H = W = 128
N = H * W       # 16384
PW = W + 1      # padded row length 129: left pad only; the next row's
                # left-pad zero doubles as this row's right pad (wraparound)
HB = 16         # band height (output rows per band)
NB = H // HB    # 8 bands
TW = (HB + 2) * PW  # 2322 cols per pad tile

WSCALE = 2048.0   # conv weight pre-scale (entries ~ (0.02)^2)
ALPHA = 2048.0    # Wproj pre-scale so Wfused lands in fp8 range
DESCALE = 1.0 / (WSCALE * ALPHA)   # exact 2^-22

# DoubleRow quads over the single A|G pad tile (A on partitions 0:64,
# G = A shifted +1 row +1 col on 64:128): (base, taps); taps =
# [(half, slot, dy, dx)]; slot stride is always 128, which in 129-wide
# padded rows steps (+1 row, -1 col) -- a walrus-legal stationary stride.
# All 9 depthwise taps fit in 3 quads; unlisted slots get zero weights.
QUADS = [
    (1, [(0, 0, -1, 0), (1, 0, 0, 1), (0, 1, 0, -1), (1, 1, 1, 0)]),
    (0, [(0, 0, -1, -1), (1, 0, 0, 0), (1, 1, 1, -1)]),
    (2, [(0, 0, -1, 1), (1, 1, 1, 1)]),
]

_PATCHED = False


def _patch_tile_drain():
    """This walrus build rejects >1 sem wait on a CTRL (Drain) instruction;
    split the TileContext tail-drain waits onto individual nops."""
    global _PATCHED
    if _PATCHED:
        return
    _PATCHED = True

    def _drain_and_barrier(self, tick_clock, wait_clock):
        nc = self.nc
        drain_inst = nc.sync.drain()
        wait_clock.add_sem_waits(
            drain_inst.ins, ScopedClock({None: tick_clock.global_clock})
        )
        si = drain_inst.ins.sync_info
        waits = list(si.on_wait or [])
        if len(waits) > 1:
            si.on_wait = waits[:1]
            for w in waits[1:]:
                nop = nc.sync.nop(nofuse=True, hint="split_wait")
                nop.ins.sync_info = bass_rust.SyncInfo(on_wait=[w], on_update=[])
        nc.all_engine_barrier()
        assert self.sems is not None
        popped = nc._tile_sem_poison_stack.pop()
        assert popped is self._sem_poison
        nc.clear_and_free_semaphores(list(self.sems.allocated().values()))
        nc.all_engine_barrier()

    tile_mod.TileContext._drain_and_barrier = _drain_and_barrier
    try:
        from concourse import tile_utils
        tile_utils.max_sbuf_usage = 208 * 1024
    except Exception:
        pass


def _split_excess_waits(nc, max_waits=1):
    """This walrus build caps sem waits per instruction encoding; hoist
    excess waits onto preceding same-engine NoOps (queues are in-order,
    so a wait satisfied on an earlier instruction orders the later one)."""
    import bass_rust as _br

    ctr = [0]
    for f in nc.m.functions:
        for blk in f.blocks:
            out = []
            for inst in blk.instructions:
                si = inst.sync_info
                waits = list(si.on_wait) if (si and si.on_wait) else []
                if len(waits) > max_waits:
                    keep = waits[:max_waits]
                    extra = waits[max_waits:]
                    si.on_wait = keep
                    for w in extra:
                        ctr[0] += 1
                        nop = _br.InstNoOp(name=f"wsplit-{ctr[0]}", ins=[], outs=[])
                        nop.engine = inst.engine
                        nop.sync_info = _br.SyncInfo(on_wait=[w], on_update=[])
                        try:
                            nc.register_instruction(nop, overwrite=True)
                        except Exception:
                            pass
                        out.append(nop)
                out.append(inst)
            blk.instructions[:] = out


def _merge_w_qk(W1, Wdw):
    """-> [128, 3 quads, 2 slots, 256] fp8 moving weights for qT/kT."""
    out = np.zeros((128, 3, 2, 256), np.float32)
    W1 = W1[:, :, 0, 0]  # [256, 64] (cout, cin)
    for qi, (_, taps) in enumerate(QUADS):
        for (half, slot, dy, dx) in taps:
            out[half * 64:(half + 1) * 64, qi, slot, :] = (
                Wdw[:, 0, 1 + dy, 1 + dx][:, None] * W1 * WSCALE
            ).T
    return out.reshape(128, 3 * 2 * 256).astype(NP8)


def _merge_w_v(W1, Wdw):
    """-> [128, 3 quads, 2 slots, 2 mb, 128] fp8 stationary weights for v."""
    out = np.zeros((128, 3, 2, 2, 128), np.float32)
    W1 = W1[:, :, 0, 0]
    for qi, (_, taps) in enumerate(QUADS):
        for (half, slot, dy, dx) in taps:
            w = (Wdw[:, 0, 1 + dy, 1 + dx][:, None] * W1 * WSCALE).T  # [64 cin, 256]
            for mb in range(2):
                out[half * 64:(half + 1) * 64, qi, slot, mb, :] = w[:, mb * 128:(mb + 1) * 128]
    return out.reshape(128, 3 * 2 * 2 * 128).astype(NP8)


def _dr2(ap, stride):
    """[p, n] AP -> [p, 2 @ stride, n] AP (DoubleRow k-tile pair)."""
    u = ap.unsqueeze(1).broadcast_to([ap.shape[0], 2, ap.shape[1]])
    a = u.ap
    a[1] = (stride, 2)
    u.ap = a
    return u


def _bf(a):
    return np.ascontiguousarray(a).astype(ml_dtypes.bfloat16)


def _build_nc(wq8, wk8, wv8, wpnat, wfusT, temp_cols):
    """Build the Bass module. wq8/wk8/wv8 [128, 1536] fp8,
    wpnat [2][128, 256] f32 (bf16-cast later), wfusT [128, 512] f32."""
    _patch_tile_drain()
    nc = bass.Bass()

    x8d = nc.declare_dram_parameter("x8", [C, N], FP8, isOutput=False)
    y8d = nc.declare_dram_parameter("y8", [C, N], FP8, isOutput=False)
    z8d = nc.declare_dram_parameter("z8", [C, N], FP8, isOutput=False)
    xd = nc.declare_dram_parameter("x", [C, N], BF16, isOutput=False)
    yd = nc.declare_dram_parameter("y", [C, N], BF16, isOutput=False)
    zd = nc.declare_dram_parameter("z", [C, N], BF16, isOutput=False)
    od = nc.declare_dram_parameter("out", [DIM, N], BF16, isOutput=True)

    wq_d = nc.inline_tensor(wq8, name="wq8")
    wk_d = nc.inline_tensor(wk8, name="wk8")
    wv_d = nc.inline_tensor(wv8, name="wv8")
    wp0_d = nc.inline_tensor(_bf(wpnat[0]), name="wpnat0")  # [128, 256]
    wp1_d = nc.inline_tensor(_bf(wpnat[1]), name="wpnat1")
    wf_d = nc.inline_tensor(_bf(wfusT), name="wfusT")       # [128, 512]
    tc0_d = nc.inline_tensor(np.ascontiguousarray(temp_cols[0]), name="tcol0")
    id_d = nc.inline_tensor(np.eye(128, dtype=ml_dtypes.bfloat16), name="ident")
    tc1_d = nc.inline_tensor(np.ascontiguousarray(temp_cols[1]), name="tcol1")

    DR = mybir.MatmulPerfMode.DoubleRow

    with TileContext(nc) as tc:
        import contextlib

        with contextlib.ExitStack() as ctx:
            wpool = ctx.enter_context(tc.tile_pool(name="wpool", bufs=1))
            vpool = ctx.enter_context(tc.tile_pool(name="vpool", bufs=1))
            pads = ctx.enter_context(tc.tile_pool(name="pads", bufs=2))
            qkp = ctx.enter_context(tc.tile_pool(name="qkp", bufs=4))
            smallp = ctx.enter_context(tc.tile_pool(name="smallp", bufs=2))
            p2p = ctx.enter_context(tc.tile_pool(name="p2p", bufs=3))

            # --- weights to SBUF ---
            wq_sb = wpool.tile([128, 3 * 512], FP8, tag="wq")
            wk_sb = wpool.tile([128, 3 * 512], FP8, tag="wk")
            wv_sb = wpool.tile([128, 3 * 512], FP8, tag="wv")
            wp_sb = [wpool.tile([128, 256], BF16, tag=f"wp{i}", name=f"wpnat{i}") for i in range(2)]
            wf_sb = wpool.tile([128, 512], BF16, tag="wf")
            wfused8 = wpool.tile([128, 512], FP8, tag="wfused8")
            ident_sb = wpool.tile([128, 128], BF16, tag="ident")
            tcol = [wpool.tile([128, 1], F32, tag=f"tc{i}", name=f"tcol{i}") for i in range(2)]

            # --- persistent state ---
            v_slab = vpool.tile([128, 2 * N], FP8, tag="vslab", name="vslab")
            p1stack = ctx.enter_context(contextlib.ExitStack())
            ps_qk = p1stack.enter_context(tc.tile_pool(name="ps_qk", bufs=3, space="PSUM"))
            ps_v = p1stack.enter_context(tc.tile_pool(name="ps_v", bufs=2, space="PSUM"))
            ps_acc = p1stack.enter_context(tc.tile_pool(name="ps_acc", bufs=1, space="PSUM"))
            acc1 = ps_acc.tile([128, 512], F32, tag="acc1")
            acc2 = ps_acc.tile([128, 256], F32, tag="acc2")
            par_all = acc1[:, 0:256]
            pgq = acc1[:, 256:512]
            pgk = acc2

            ins8 = [x8d, y8d, z8d]

            # ================= pass 1: bands =================
            copy_engs = [
                lambda o, i: nc.scalar.copy(o, i),
                lambda o, i: nc.vector.tensor_copy(out=o, in_=i),
            ]
            cat2 = None
            for b in range(NB):
                lr0 = 1 if b == 0 else 0
                nr = (HB + 2) - (1 if b == 0 else 0) - (1 if b == NB - 1 else 0)
                ir0 = max(0, HB * b - 1)

                srcs = []  # per input: A|G pad tile
                tile_engs = [
                    (nc.sync, nc.gpsimd),
                    (nc.gpsimd, nc.sync),
                    (nc.sync, nc.gpsimd),
                ]
                for ti, td in enumerate(ins8):
                    nm = "xyz"[ti]
                    eA, eG = tile_engs[ti]
                    AG = pads.tile([128, TW], FP8, tag=f"{nm}AG")
                    src_img = td[:].rearrange("p (r c) -> p r c", c=W)[
                        :, ir0: ir0 + nr, :
                    ]
                    viewA = AG[0:64, :].rearrange("p (r c) -> p r c", c=PW)
                    viewG = AG[64:128, :].rearrange("p (r c) -> p r c", c=PW)
                    # A: [0pad, img row]; G[rr, 0:128] = img[row+1, 0:128],
                    # G col 128 = 0 (the A-plane wraparound supplies right
                    # pads; G needs its own at col 128).
                    nc.gpsimd.memset(viewA[:, :, 0:1], 0.0)
                    nc.gpsimd.memset(viewG[:, :, 128:129], 0.0)
                    if b == 0:
                        nc.gpsimd.memset(viewA[:, 0:1, :], 0.0)
                    if b == NB - 1:
                        nc.gpsimd.memset(viewA[:, HB + 1: HB + 2, :], 0.0)
                        nc.gpsimd.memset(viewG[:, HB: HB + 1, :], 0.0)
                    if b == 0 and ti == 0:
                        nc.scalar.dma_start(out=wq_sb, in_=wq_d[:])
                    eA.dma_start(
                        out=viewA[:, lr0: lr0 + nr, 1: 1 + W], in_=src_img
                    )
                    if b == 0:
                        eG.dma_start(
                            out=viewG[:, 0: nr, 0: W], in_=src_img
                        )
                    elif b == NB - 1:
                        eG.dma_start(
                            out=viewG[:, 0: nr - 1, 0: W], in_=src_img[:, 1:, :]
                        )
                    else:
                        eG.dma_start(
                            out=viewG[:, 0: nr - 1, 0: W], in_=src_img[:, 1:, :]
                        )
                    if b == 0 and ti == 0:
                        nc.scalar.dma_start(out=wk_sb, in_=wk_d[:])
                        nc.scalar.dma_start(out=wv_sb, in_=wv_d[:])
                    srcs.append(AG)

                # qT/kT convs + gram per output row (pairs of 2 rows)
                for hl in range(HB):
                    g = HB * b + hl
                    base = hl * PW
                    pqk_t = ps_qk.tile([128, 512], F32, tag="pqk")
                    pk_t = pqk_t[:, 0:256]
                    pq_t = pqk_t[:, 256:512]
                    for which, (w_sb, p_t) in enumerate(
                        ((wq_sb, pq_t), (wk_sb, pk_t))
                    ):
                        AG = srcs[which]
                        for qi, (qbase, _) in enumerate(QUADS):
                            lhsT = _dr2(
                                AG[0:128, base + qbase: base + qbase + 128],
                                128,
                            )
                            rhs = w_sb[0:128, qi * 512:(qi + 1) * 512].rearrange(
                                "p (a c) -> p a c", a=2
                            )
                            nc.tensor.matmul(
                                p_t, lhsT=lhsT, rhs=rhs,
                                start=(qi == 0), stop=(qi == 2), perf_mode=DR,
                            )
                    slot = hl % 2
                    if slot == 0:
                        cat2 = qkp.tile([128, 1024], FP8, tag="cat2")
                    copy_engs[(g // 2) % 2](cat2[:, slot * 512:(slot + 1) * 512], pqk_t)
                    if slot == 1:
                        pfirst, plast = g == 1, g == H - 1
                        for mb in range(2):
                            qsl = _dr2(cat2[0:128, 256 + mb * 128: 384 + mb * 128], 512)
                            ksl = _dr2(cat2[0:128, mb * 128: mb * 128 + 128], 512)
                            nc.tensor.matmul(
                                par_all[:, bass.ds(mb * 128, 128)],
                                lhsT=qsl, rhs=ksl,
                                start=pfirst, stop=plast,
                                skip_group_check=True, perf_mode=DR,
                            )
                            nc.tensor.matmul(
                                pgq[:, bass.ds(mb * 128, 128)],
                                lhsT=qsl, rhs=qsl,
                                start=pfirst, stop=plast,
                                skip_group_check=True, perf_mode=DR,
                            )
                            nc.tensor.matmul(
                                pgk[:, bass.ds(mb * 128, 128)],
                                lhsT=ksl, rhs=ksl,
                                start=pfirst, stop=plast,
                                skip_group_check=True, perf_mode=DR,
                            )

                # v conv (natural layout), 4 chunks of 4 rows
                zAG = srcs[2]
                for cc in range(HB // 4):
                    hl0 = 4 * cc
                    for mb in range(2):
                        pv_t = ps_v.tile([128, 512], F32, tag="pv")
                        for r in range(4):
                            rbase = (hl0 + r) * PW
                            for qi, (qbase, _) in enumerate(QUADS):
                                w0 = qi * 512 + mb * 128
                                lhsT = _dr2(wv_sb[0:128, w0: w0 + 128], 256)
                                mv = _dr2(
                                    zAG[0:128, rbase + qbase: rbase + qbase + 128],
                                    128,
                                )
                                nc.tensor.matmul(
                                    pv_t[:, r * 128:(r + 1) * 128],
                                    lhsT=lhsT, rhs=mv,
                                    start=(qi == 0), stop=(qi == 2), perf_mode=DR,
                                )
                        dst = v_slab[:, bass.ds(mb * N + (HB * b + hl0) * W, 512)]
                        copy_engs[(b * 4 + cc + mb) % 2](dst, pv_t)

            nc.sync.dma_start(out=wp_sb[0], in_=wp0_d[:])
            nc.sync.dma_start(out=wp_sb[1], in_=wp1_d[:])
            nc.sync.dma_start(out=wf_sb, in_=wf_d[:])
            nc.sync.dma_start(out=tcol[0], in_=tc0_d[:])
            nc.sync.dma_start(out=tcol[1], in_=tc1_d[:])
            nc.sync.dma_start(out=ident_sb, in_=id_d[:])

            # ================= phase 1.5: softmax on [256, 32] =================
            ar_sb = [smallp.tile([128, 128], F32, tag=f"arsb{mb}", name=f"arsb{mb}") for mb in range(2)]
            nc.scalar.copy(ar_sb[0], par_all[:, 0:128])
            nc.scalar.copy(ar_sb[1], par_all[:, 128:256])
            bd = [smallp.tile([128, 128], BF16, tag=f"bd{mb}", name=f"bdiag{mb}") for mb in range(2)]
            for mb in range(2):
                scr = smallp.tile([128, 128], F32, tag="scr")
                rnq_c = smallp.tile([128, 1], F32, tag="rnq")
                rnk_c = smallp.tile([128, 1], F32, tag="rnk")
                for g_ps, dst in ((pgq, rnq_c), (pgk, rnk_c)):
                    ssum = smallp.tile([128, 1], F32, tag="ssum")
                    nc.vector.tensor_mul(scr, g_ps[:, bass.ds(mb * 128, 128)], ident_sb)
                    nc.vector.reduce_sum(out=ssum, in_=scr, axis=mybir.AxisListType.X)
                    nc.scalar.sqrt(ssum, ssum)
                    nc.vector.tensor_scalar_max(ssum, ssum, 1e-12)
                    nc.vector.reciprocal(dst, ssum)
                rnqt = smallp.tile([128, 1], F32, tag="rnqt")
                nc.vector.tensor_mul(rnqt, rnq_c, tcol[mb])

                hd = smallp.tile([128, 32], F32, tag="hd")
                for i in range(4):
                    nc.vector.tensor_copy(
                        out=hd[32 * i: 32 * (i + 1), :],
                        in_=ar_sb[mb][32 * i: 32 * (i + 1), bass.ds(32 * i, 32)],
                    )
                hds = smallp.tile([128, 32], F32, tag="hds")
                nc.scalar.activation(
                    hds, hd, mybir.ActivationFunctionType.Copy, bias=0.0, scale=rnqt
                )
                hdT = smallp.tile([128, 32], F32, tag="hdT")
                nc.vector.transpose(hdT, hds)
                hdTs = smallp.tile([128, 32], F32, tag="hdTs")
                nc.scalar.activation(
                    hdTs, hdT, mybir.ActivationFunctionType.Copy, bias=0.0, scale=rnk_c
                )
                hd3 = smallp.tile([128, 32], F32, tag="hd3")
                nc.vector.transpose(hd3, hdTs)
                nmx = smallp.tile([128, 1], F32, tag="nmx")
                nc.vector.reduce_max(
                    out=nmx, in_=hd3, axis=mybir.AxisListType.X, negate=True
                )
                ex = smallp.tile([128, 32], F32, tag="ex")
                nc.scalar.activation(
                    ex, hd3, mybir.ActivationFunctionType.Exp, bias=nmx, scale=1.0
                )
                sm = smallp.tile([128, 1], F32, tag="sm")
                nc.vector.reduce_sum(out=sm, in_=ex, axis=mybir.AxisListType.X)
                rsm = smallp.tile([128, 1], F32, tag="rsm")
                nc.vector.reciprocal(rsm, sm)
                Pt = smallp.tile([128, 32], F32, tag="Pt")
                nc.scalar.activation(
                    Pt, ex, mybir.ActivationFunctionType.Copy, bias=0.0, scale=rsm
                )
                PtT = smallp.tile([128, 32], F32, tag="PtT")
                nc.vector.transpose(PtT, Pt)
                nc.gpsimd.memset(bd[mb], 0.0)
                for i in range(4):
                    nc.vector.tensor_copy(
                        out=bd[mb][32 * i: 32 * (i + 1), bass.ds(32 * i, 32)],
                        in_=PtT[32 * i: 32 * (i + 1), :],
                    )

            # Wfused^T[j, o] = sum_i attn[i, j] * (Wproj^T * ALPHA)[i, o]
            for mb in range(2):
                pwf = ps_qk.tile([128, 512], F32, tag="pqk")
                nc.tensor.matmul(
                    pwf[:, 0:256], lhsT=bd[mb], rhs=wp_sb[mb][:], start=True, stop=True
                )
                nc.scalar.copy(wfused8[:, mb * 256:(mb + 1) * 256], pwf[:, 0:256])

            # ================= pass 2: proj + fusion =================
            p1stack.close()
            ps_po = ctx.enter_context(tc.tile_pool(name="ps_po", bufs=4, space="PSUM"))

            def p2_load(ch):
                n0 = 512 * ch
                xy_t = p2p.tile([128, 512], BF16, tag="xy", name="xy_t")
                z_t = p2p.tile([64, 512], BF16, tag="zt", name="z_t")
                nc.gpsimd.dma_start(out=xy_t[0:64, :], in_=xd[:, bass.ds(n0, 512)])
                nc.gpsimd.dma_start(out=xy_t[64:128, :], in_=yd[:, bass.ds(n0, 512)])
                nc.sync.dma_start(out=z_t, in_=zd[:, bass.ds(n0, 512)])
                return xy_t, z_t

            def p2_proj(ch, xy_t, z_t):
                n0 = 512 * ch
                for mb in range(2):
                    po = ps_po.tile([128, 512], F32, tag="po", name="po")
                    nc.tensor.matmul(
                        po,
                        lhsT=_dr2(wfused8[0:128, mb * 128: mb * 128 + 128], 256),
                        rhs=_dr2(v_slab[0:128, n0: n0 + 512], N),
                        start=True, stop=False, perf_mode=DR,
                        skip_group_check=True,
                    )
                    nc.tensor.matmul(
                        po,
                        lhsT=wf_sb[0:128, bass.ds(mb * 128, 128)],
                        rhs=xy_t,
                        start=False, stop=False,
                        skip_group_check=True,
                    )
                    nc.tensor.matmul(
                        po,
                        lhsT=wf_sb[0:64, bass.ds(256 + mb * 128, 128)],
                        rhs=z_t,
                        start=False, stop=True,
                        skip_group_check=True,
                    )
                    o_t = p2p.tile([128, 512], BF16, tag=f"ot{mb}", name="o_t")
                    if mb == 0:
                        nc.scalar.activation(
                            o_t, po, mybir.ActivationFunctionType.Copy,
                            bias=0.0, scale=DESCALE,
                        )
                    else:
                        nc.vector.tensor_scalar_mul(o_t, po, DESCALE)
                    (nc.sync if mb == 0 else nc.scalar).dma_start(
                        out=od[bass.ds(mb * 128, 128), bass.ds(n0, 512)], in_=o_t
                    )

            pending = None
            for ch in range(32):
                cur = p2_load(ch)
                if pending is not None:
                    p2_proj(ch - 1, *pending)
                pending = cur
            p2_proj(31, *pending)

    _split_excess_waits(nc)
    return nc


def _prep_weights(inputs):
    wq8 = _merge_w_qk(np.asarray(inputs["Wq"], np.float32), np.asarray(inputs["Wq_dw"], np.float32))
    wk8 = _merge_w_qk(np.asarray(inputs["Wk"], np.float32), np.asarray(inputs["Wk_dw"], np.float32))
    wv8 = _merge_w_v(np.asarray(inputs["Wv"], np.float32), np.asarray(inputs["Wv_dw"], np.float32))

    wproj = np.asarray(inputs["Wproj"], np.float32)[:, :, 0, 0]  # [o, i]
    wpnat = [np.ascontiguousarray(wproj.T[i * 128:(i + 1) * 128] * ALPHA) for i in range(2)]

    wfus = np.asarray(inputs["Wfus"], np.float32)[:, :, 0, 0]  # [256, 192]
    wfusT = np.zeros((128, 512), np.float32)
    wfusT[:, 0:256] = wfus[:, 0:128].T * (WSCALE * ALPHA)
    wfusT[0:64, 256:512] = wfus[:, 128:192].T * (WSCALE * ALPHA)

    temp = np.asarray(inputs["temperature"], np.float32).reshape(HEADS)
    tfull = np.repeat(temp, 32).astype(np.float32)
    temp_cols = [tfull[0:128].reshape(128, 1), tfull[128:256].reshape(128, 1)]
    return wq8, wk8, wv8, wpnat, wfusT, temp_cols


def kernel(**inputs):
    x = np.asarray(inputs["x"], np.float32)
    y = np.asarray(inputs["y"], np.float32)
    z = np.asarray(inputs["z"], np.float32)
    B = x.shape[0]
    assert B == 8

    nc = _build_nc(*_prep_weights(inputs))

    in_maps = []
    for i in range(B):
        in_maps.append(
            {
                "x8": x[i].reshape(C, N).astype(NP8),
                "y8": y[i].reshape(C, N).astype(NP8),
                "z8": z[i].reshape(C, N).astype(NP8),
                "x": _bf(x[i].reshape(C, N)),
                "y": _bf(y[i].reshape(C, N)),
                "z": _bf(z[i].reshape(C, N)),
            }
        )
    res = run_bass_kernel_spmd(nc, in_maps, list(range(8)))
    out = np.stack(
        [np.asarray(res.results[i]["out"], np.float32).reshape(DIM, H, W) for i in range(B)]
    )
    return out
